# revision 36
# baseline (speedup 1.0000x reference)
"""TransformerConv GNN block (nn_Block_28192165331060) on 8 Trainium2 NeuronCores.

v2 design:
  - Nodes sharded contiguously (6250/core); edges partitioned by dst owner.
  - Algebraic refold: score_e = w[dst].x[src] with w = x @ (Wq Wk^T) + Wk bq
    (dst-only terms cancel in softmax), and agg = (sum p*x[src])/den @ Wv
    folded into the output projection (WVO = Wv @ WO). So the edge phase
    gathers RAW x rows only - no k/v tables.
  - Fixed windows of 128 consecutive dst nodes (= node tiles): w_win comes
    from a plain matmul, aggregate flushes stay in SBUF - the ONLY indirect
    ops are the x-row gathers.
  - Per window: 2x InstDMAGatherAnt (even/odd src halves so indices fit
    int16), bf16 rows (256B), striped across 4 SWDGE queues.
  - Inner 128-edge tile: onehot built via DVE is_equal in BOTH orientations
    (column rel + broadcast-DMA'd row rel) -> zero PE transposes; score via
    fused tensor_tensor_reduce; p = exp on ACT; scat = fused
    is_equal*p tensor_scalar; two PE accum matmuls (agg, den).
  - Dense tail (skip/O proj, residuals, BN1, FFN, BN2) in transposed
    [feat, node] space; BN stats AllReduce'd across the 8 cores.
"""

import math

import numpy as np
from ml_dtypes import bfloat16

N_NODES = 50000
D = 128
NC = 8
NL = N_NODES // NC          # 6250 nodes per core
NLP = 6272                  # padded local nodes (49 * 128)
NT = NLP // 128             # 49 windows (= node tiles) per core
NFULL = 50048               # padded full nodes (391 * 128)
NHALF = NFULL // 2          # 25024 rows per parity table
EPS = 1e-5


# ---------------------------------------------------------------------------
# Host-side preprocessing
# ---------------------------------------------------------------------------

def host_prep(x, edge_index, weights, cfg):
    """Build per-core device input arrays. Returns (in_maps, tev, tod)."""
    nl = cfg["nl"]; nlp = cfg["nlp"]; nc_ = cfg["nc"]; nt = nlp // 128

    x = np.asarray(x, dtype=np.float32)
    src = np.asarray(edge_index[0], dtype=np.int64)
    dst = np.asarray(edge_index[1], dtype=np.int64)

    W = {k: np.asarray(v, dtype=np.float32) for k, v in weights.items()}
    A_ = (W["Wq"] @ W["Wk"].T).astype(np.float32)
    brow = (W["Wk"] @ W["bq"]).astype(np.float32)           # [128]
    b_brow = np.broadcast_to(np.concatenate([brow] * 4)[None, :], (128, 512)).astype(np.float32).copy()
    WSO = (W["Ws"] @ W["WO"]).astype(np.float32)
    WVO = (W["Wv"] @ W["WO"]).astype(np.float32)
    beff_col = (((W["bs"] + W["bv"]) @ W["WO"]) + W["bO"]).astype(np.float32).reshape(128, 1)

    bcols = np.zeros((128, 8), dtype=np.float32)
    bcols[:, 0] = W["b1"][0:128]
    bcols[:, 1] = W["b1"][128:256]
    bcols[:, 2] = W["b2"]
    bcols[:, 3] = W["g1"]
    bcols[:, 4] = W["be1"]
    bcols[:, 5] = W["g2"]
    bcols[:, 6] = W["be2"]

    iota_f = np.tile(np.arange(128, dtype=np.float32)[None, :], (128, 1)).copy()
    iota_bf = iota_f.astype(bfloat16)
    iota_col = np.arange(128, dtype=np.float32).reshape(128, 1).copy()
    ones_col = np.ones((128, 1), dtype=bfloat16)
    id_f32 = np.eye(128, dtype=np.float32)

    x_pad = np.zeros((NFULL, 128), dtype=np.float32)
    x_pad[:N_NODES] = x
    x_ev = np.ascontiguousarray(x_pad[0::2]).astype(bfloat16)
    x_od = np.ascontiguousarray(x_pad[1::2]).astype(bfloat16)

    # --- per-core edge partition + window stats -----------------------------
    cores = []
    tev_need, tod_need = 1, 1
    owner = dst // nl
    for c in range(nc_):
        m = owner == c
        s_c = src[m]
        dl = dst[m] - c * nl
        w_of = (dl // 128).astype(np.int64)
        rel = (dl % 128).astype(np.int64)
        par = (s_c & 1).astype(np.int64)
        order = np.argsort(w_of * 2 + par, kind="stable")
        s_c, rel, w_of, par = s_c[order], rel[order], w_of[order], par[order]
        cores.append((s_c, rel, w_of, par))
        for w in range(nt):
            mm = w_of == w
            ne = int((par[mm] == 0).sum())
            no = int((par[mm] == 1).sum())
            tev_need = max(tev_need, (ne + 127) // 128)
            tod_need = max(tod_need, (no + 127) // 128)
    tev, tod = tev_need, tod_need
    T = tev + tod
    cap_ev, cap_od = tev * 128, tod * 128

    def wrap16(arr):
        # index j -> [j % 16, j // 16], replicated to 128 partitions
        a = arr.reshape(-1, 16).T  # [16, cap/16]
        return np.tile(a, (8, 1))

    shared = {
        "x_ev": x_ev, "x_od": x_od,
        "A_": A_, "b_brow": b_brow, "WSO": WSO, "WVO_": WVO,
        "beff_col": beff_col, "W1_": W["W1"].copy(), "W2_": W["W2"].copy(),
        "bcols": bcols, "iota_f": iota_f, "iota_bf": iota_bf, "iota_col": iota_col,
        "ones_col": ones_col, "id_f32": id_f32,
    }

    npair = (nt + 3) // 4
    in_maps = []
    for c in range(nc_):
        s_c, rel, w_of, par = cores[c]
        meta_idx = np.zeros((nt, 128, (cap_ev + cap_od) // 16), dtype=np.int16)
        meta_rel = np.full((nt, 128, T), -1.0, dtype=np.float32)
        relrow = np.full((nt, T * 128), -1.0, dtype=np.float32)
        for w in range(nt):
            mm = w_of == w
            sw, rw, pw = s_c[mm], rel[mm], par[mm]
            evm = pw == 0
            idx_ev = np.zeros(cap_ev, dtype=np.int16)
            rel_ev = np.full(cap_ev, -1.0, dtype=np.float32)
            ne = int(evm.sum())
            idx_ev[:ne] = (sw[evm] // 2).astype(np.int16)
            rel_ev[:ne] = rw[evm].astype(np.float32)
            idx_od = np.zeros(cap_od, dtype=np.int16)
            rel_od = np.full(cap_od, -1.0, dtype=np.float32)
            no = int((~evm).sum())
            idx_od[:no] = (sw[~evm] // 2).astype(np.int16)
            rel_od[:no] = rw[~evm].astype(np.float32)

            meta_idx[w, :, : cap_ev // 16] = wrap16(idx_ev)
            meta_idx[w, :, cap_ev // 16 :] = wrap16(idx_od)
            # slot (tile tt, partition p): even tiles 0..tev-1, odd after
            meta_rel[w, :, :tev] = rel_ev.reshape(tev, 128).T
            meta_rel[w, :, tev:] = rel_od.reshape(tod, 128).T
            relrow[w, : cap_ev] = rel_ev
            relrow[w, cap_ev:] = rel_od

        x_loc_pad = np.zeros((nlp, 128), dtype=np.float32)
        x_loc_pad[:nl] = x[c * nl : (c + 1) * nl]
        xT_loc = np.ascontiguousarray(x_loc_pad.T)

        ic_ = (cap_ev + cap_od) // 16
        mi2 = np.zeros((npair, 128, 4 * ic_), dtype=np.int16)
        mr2 = np.full((npair, 128, 4 * T), -1.0, dtype=np.float32)
        rr2 = np.full((npair, 4 * T * 128), -1.0, dtype=np.float32)
        for k in range(npair):
            for w in range(4):
                w0 = 4 * k + w
                if w0 >= nt:
                    break
                mi2[k, :, w * ic_ : (w + 1) * ic_] = meta_idx[w0]
                mr2[k, :, w * T : (w + 1) * T] = meta_rel[w0]
                rr2[k, w * T * 128 : (w + 1) * T * 128] = relrow[w0]

        im = dict(shared)
        im["xT_loc"] = xT_loc
        im["xT_loc_bf"] = xT_loc.astype(bfloat16)
        im["meta_idx"] = mi2
        im["meta_rel"] = mr2
        im["relrow"] = rr2.astype(bfloat16)
        in_maps.append(im)
    return in_maps, tev, tod


# ---------------------------------------------------------------------------
# Device kernel
# ---------------------------------------------------------------------------

def build_kernel(cfg, tev, tod, debug=False):
    import concourse.bacc as bacc
    import concourse.tile as tile
    import concourse.mybir as mybir
    from concourse import bass
    from concourse import library_config

    dt = mybir.dt
    nlp = cfg["nlp"]; nl = cfg["nl"]
    nt = nlp // 128
    T = tev + tod
    cap_ev, cap_od = tev * 128, tod * 128
    ic = (cap_ev + cap_od) // 16
    inv_sqrt_d = 1.0 / math.sqrt(128.0)
    inv_n = 1.0 / float(cfg["n_nodes"])

    nc = bacc.Bacc(None, target_bir_lowering=False, debug=False,
                   num_swdge_queues=4)

    # ---- I/O ----
    x_ev = nc.declare_dram_parameter("x_ev", [NHALF, 128], dt.bfloat16, isOutput=False)
    x_od = nc.declare_dram_parameter("x_od", [NHALF, 128], dt.bfloat16, isOutput=False)
    xT_loc = nc.declare_dram_parameter("xT_loc", [128, nlp], dt.float32, isOutput=False)
    npair_ = (nt + 3) // 4
    meta_idx = nc.declare_dram_parameter("meta_idx", [npair_, 128, 4 * ic], dt.int16, isOutput=False)
    relrow = nc.declare_dram_parameter("relrow", [npair_, 4 * T * 128], dt.bfloat16, isOutput=False)
    meta_rel = nc.declare_dram_parameter("meta_rel", [npair_, 128, 4 * T], dt.float32, isOutput=False)
    A_ = nc.declare_dram_parameter("A_", [128, 128], dt.float32, isOutput=False)
    b_brow = nc.declare_dram_parameter("b_brow", [128, 512], dt.float32, isOutput=False)
    WSO = nc.declare_dram_parameter("WSO", [128, 128], dt.float32, isOutput=False)
    WVO_ = nc.declare_dram_parameter("WVO_", [128, 128], dt.float32, isOutput=False)
    beff_col = nc.declare_dram_parameter("beff_col", [128, 1], dt.float32, isOutput=False)
    W1_ = nc.declare_dram_parameter("W1_", [128, 256], dt.float32, isOutput=False)
    W2_ = nc.declare_dram_parameter("W2_", [256, 128], dt.float32, isOutput=False)
    bcols = nc.declare_dram_parameter("bcols", [128, 8], dt.float32, isOutput=False)
    iota_f = nc.declare_dram_parameter("iota_f", [128, 128], dt.float32, isOutput=False)
    iota_bf = nc.declare_dram_parameter("iota_bf", [128, 128], dt.bfloat16, isOutput=False)
    xT_loc_bf = nc.declare_dram_parameter("xT_loc_bf", [128, nlp], dt.bfloat16, isOutput=False)
    iota_col = nc.declare_dram_parameter("iota_col", [128, 1], dt.float32, isOutput=False)
    ones_col = nc.declare_dram_parameter("ones_col", [128, 1], dt.bfloat16, isOutput=False)
    id_f32 = nc.declare_dram_parameter("id_f32", [128, 128], dt.float32, isOutput=False)
    yT_out = nc.declare_dram_parameter("yT_out", [128, nlp], dt.float32, isOutput=True)

    # ---- internal DRAM (collectives) ----
    st1_in = nc.dram_tensor("st1_in", [128, 2], dt.float32)
    st1_out = nc.dram_tensor("st1_out", [128, 2], dt.float32, addr_space="Shared")
    st2_in = nc.dram_tensor("st2_in", [128, 2], dt.float32)
    st2_out = nc.dram_tensor("st2_out", [128, 2], dt.float32, addr_space="Shared")

    rg = [list(range(cfg["nc"]))]
    pad0 = nl % 128  # 106: first pad column in the last tile

    with tile.TileContext(nc) as tc:
        with (
            tc.tile_pool(name="const", bufs=1) as constp,
            tc.tile_pool(name="meta", bufs=4) as metap,
            tc.tile_pool(name="rbc", bufs=3) as rbcp,
            tc.tile_pool(name="xg", bufs=4) as xgp,
            tc.tile_pool(name="xt", bufs=2) as xtp,
            tc.tile_pool(name="edge", bufs=6) as edgep,
            tc.tile_pool(name="otp", bufs=2) as otp,
            tc.tile_pool(name="small", bufs=8) as smallp,
            tc.tile_pool(name="win", bufs=3) as winp,
            tc.tile_pool(name="hold", bufs=1) as holdp,
            tc.tile_pool(name="p2", bufs=3) as p2p,
            tc.tile_pool(name="ps5", bufs=2, space="PSUM") as ps512p,
            tc.tile_pool(name="psw", bufs=2, space="PSUM") as psp,
            tc.tile_pool(name="psa", bufs=2, space="PSUM") as psap,
            tc.tile_pool(name="psd", bufs=1, space="PSUM") as psdp,
        ):
            nc.gpsimd.load_library(library_config.mlp)

            # ---------------- constants ----------------
            w_A = constp.tile([128, 128], dt.float32)
            nc.sync.dma_start(w_A[:], A_[:, :])
            c_brow = constp.tile([128, 512], dt.float32)
            nc.sync.dma_start(c_brow[:], b_brow[:, :])
            w_so = constp.tile([128, 128], dt.float32)
            nc.sync.dma_start(w_so[:], WSO[:, :])
            w_vo = constp.tile([128, 128], dt.float32)
            nc.sync.dma_start(w_vo[:], WVO_[:, :])
            c_beff = constp.tile([128, 1], dt.float32)
            nc.sync.dma_start(c_beff[:], beff_col[:, :])
            w_1 = constp.tile([128, 256], dt.float32)
            nc.sync.dma_start(w_1[:], W1_[:, :])
            w_2 = constp.tile([128, 256], dt.float32)
            nc.sync.dma_start(w_2[:, 0:128], W2_[0:128, :])
            nc.sync.dma_start(w_2[:, 128:256], W2_[128:256, :])
            bc = constp.tile([128, 8], dt.float32)
            nc.sync.dma_start(bc[:], bcols[:, :])
            c_iota = constp.tile([128, 128], dt.float32)
            nc.sync.dma_start(c_iota[:], iota_f[:, :])
            c_iotab = constp.tile([128, 128], dt.bfloat16)
            nc.sync.dma_start(c_iotab[:], iota_bf[:, :])
            c_iotac = constp.tile([128, 1], dt.float32)
            nc.sync.dma_start(c_iotac[:], iota_col[:, :])
            c_ones = constp.tile([128, 1], dt.bfloat16)
            nc.sync.dma_start(c_ones[:], ones_col[:, :])
            c_id = constp.tile([128, 128], dt.float32)
            nc.sync.dma_start(c_id[:], id_f32[:, :])
            c_idb = constp.tile([128, 128], dt.bfloat16)
            nc.vector.tensor_scalar_mul(c_idb[:], c_id[:], 1.0)
            w_Ab = constp.tile([128, 128], dt.bfloat16)
            nc.vector.tensor_scalar_mul(w_Ab[:], w_A[:], 1.0)
            w_vob = constp.tile([128, 128], dt.bfloat16)
            nc.vector.tensor_scalar_mul(w_vob[:], w_vo[:], 1.0)
            w_1b = constp.tile([128, 256], dt.bfloat16)
            nc.vector.tensor_scalar_mul(w_1b[:], w_1[:], 1.0)
            w_2b = constp.tile([128, 256], dt.bfloat16)
            nc.vector.tensor_scalar_mul(w_2b[:], w_2[:], 1.0)

            h3hold = holdp.tile([128, nlp], dt.float32, tag="h3hold")
            h5hold = holdp.tile([128, nlp], dt.float32, tag="h5hold")
            sum1 = holdp.tile([128, npair_], dt.float32, tag="sum1")
            sq1 = holdp.tile([128, npair_], dt.float32, tag="sq1")
            n_p2 = (nt - 1) // 2 + 1
            sum2 = holdp.tile([128, n_p2], dt.float32, tag="sum2")
            sq2 = holdp.tile([128, n_p2], dt.float32, tag="sq2")

            # ---------------- phase 1: window pairs ----------------
            qctr = [0]
            for k in range(npair_):
                pw = min(4, nt - 4 * k)
                mi = metap.tile([128, 4 * ic], dt.int16, tag="mi")
                nc.sync.dma_start(mi[:, 0 : pw * ic], meta_idx[k, :, 0 : pw * ic])
                mr = metap.tile([128, 4 * T], dt.float32, tag="mr")
                nc.sync.dma_start(mr[:, 0 : pw * T], meta_rel[k, :, 0 : pw * T])
                rbc = rbcp.tile([128, 4 * T * 128], dt.bfloat16, tag="rbc")
                nc.sync.dma_start(
                    rbc[:, 0 : pw * T * 128],
                    relrow[k, 0 : pw * T * 128].partition_broadcast(128),
                )
                xt = xtp.tile([128, 512], dt.float32, tag="xt")
                nc.sync.dma_start(xt[:, 0 : pw * 128],
                                  xT_loc[:, 4 * k * 128 : (4 * k + pw) * 128])
                xtb = xtp.tile([128, 512], dt.bfloat16, tag="xtb")
                nc.sync.dma_start(xtb[:, 0 : pw * 128],
                                  xT_loc_bf[:, 4 * k * 128 : (4 * k + pw) * 128])

                wps_t = ps512p.tile([128, 512], dt.float32, tag="w512")
                for w in range(pw):
                    nc.tensor.matmul(wps_t[:, w * 128 : (w + 1) * 128],
                                     lhsT=xtb[:, w * 128 : (w + 1) * 128],
                                     rhs=w_Ab[:], start=True, stop=True)
                w_win = winp.tile([128, 512], dt.bfloat16, tag="w_win")
                nc.vector.tensor_tensor(
                    out=w_win[:, 0 : pw * 128], in0=wps_t[:, 0 : pw * 128],
                    in1=c_brow[:, 0 : pw * 128], op=mybir.AluOpType.add
                )

                tr_pair = ps512p.tile([128, 512], dt.float32, tag="w512")
                accs = []
                for w in range(pw):
                    # gathers for window 4k+w
                    xg_ev = xgp.tile([128, tev, 128], dt.bfloat16, tag="xg_ev")
                    xg_od = xgp.tile([128, tod, 128], dt.bfloat16, tag="xg_od")
                    for (xg_t, x_t, cap, col0) in (
                        (xg_ev, x_ev, cap_ev, w * ic),
                        (xg_od, x_od, cap_od, w * ic + cap_ev // 16),
                    ):
                        ntile = cap // 128
                        nsplit = (ntile + 7) // 8
                        t0 = 0
                        while t0 * 128 < cap:
                            nch = min((ntile + nsplit - 1) // nsplit, ntile - t0)
                            nidx = nch * 128
                            nc.gpsimd.dma_gather(
                                xg_t[:, t0 : t0 + nch, :], x_t[:, :],
                                mi[:, col0 + t0 * 8 : col0 + (t0 + nch) * 8],
                                nidx, nidx, 128, queue_num=qctr[0] % 4,
                            )
                            qctr[0] += 1
                            t0 += nch

                    acc = psap.tile([128, 128], dt.float32, tag="acc")
                    accd = psdp.tile([128, 1], dt.float32, tag="accd")
                    accs.append((acc, accd))
                    ot_all = otp.tile([128, T * 128], dt.bfloat16, tag="ot_all")
                    nc.vector.tensor_scalar(
                        out=ot_all[:],
                        in0=rbc[:, w * T * 128 : (w + 1) * T * 128],
                        scalar1=c_iotac[:],
                        scalar2=None,
                        op0=mybir.AluOpType.is_equal,
                    )
                    for tt in range(T):
                        xg = xg_ev[:, tt, :] if tt < tev else xg_od[:, tt - tev, :]
                        wdst = psp.tile([128, 128], dt.float32, tag="wdst")
                        nc.tensor.matmul(wdst[:],
                                         lhsT=ot_all[:, tt * 128 : (tt + 1) * 128],
                                         rhs=w_win[:, w * 128 : (w + 1) * 128],
                                         start=True, stop=True)
                        junk = edgep.tile([128, 128], dt.bfloat16, tag="junk")
                        nc.vector.tensor_tensor(
                            out=junk[:], in0=wdst[:], in1=xg, op=mybir.AluOpType.mult
                        )
                        junk2 = edgep.tile([128, 128], dt.bfloat16, tag="junk2")
                        score = smallp.tile([128, 1], dt.float32, tag="score")
                        nc.scalar.activation(
                            junk2[:], junk[:], mybir.ActivationFunctionType.Copy,
                            accum_out=score[:],
                        )
                        pcol = smallp.tile([128, 1], dt.float32, tag="pcol")
                        nc.scalar.activation(
                            pcol[:], score[:], mybir.ActivationFunctionType.Exp,
                            scale=inv_sqrt_d,
                        )
                        scat = edgep.tile([128, 128], dt.bfloat16, tag="scat")
                        nc.vector.tensor_scalar(
                            out=scat[:], in0=c_iotab[:],
                            scalar1=mr[:, w * T + tt : w * T + tt + 1],
                            scalar2=pcol[:],
                            op0=mybir.AluOpType.is_equal, op1=mybir.AluOpType.mult,
                        )
                        nc.tensor.matmul(acc[:], lhsT=scat[:], rhs=xg,
                                         start=(tt == 0), stop=(tt == T - 1))
                        nc.tensor.matmul(accd[:], lhsT=scat[:], rhs=c_ones[:],
                                         start=(tt == 0), stop=(tt == T - 1))

                    # per-window: normalize + transpose into the pair tile
                    dsafe = smallp.tile([128, 1], dt.float32, tag="dsafe")
                    nc.vector.tensor_scalar_max(dsafe[:], accd[:], 1e-30)
                    rec = smallp.tile([128, 1], dt.float32, tag="rec")
                    nc.vector.reciprocal(rec[:], dsafe[:])
                    hat = winp.tile([128, 128], dt.float32, tag="hat")
                    nc.vector.tensor_scalar(
                        out=hat[:], in0=acc[:], scalar1=rec[:], scalar2=None,
                        op0=mybir.AluOpType.mult,
                    )
                    nc.tensor.transpose(tr_pair[:, w * 128 : (w + 1) * 128],
                                        in_=hat[:], identity=c_id[:])

                # ---- pair post-attention ----
                hatT = winp.tile([128, 512], dt.float32, tag="hatT")
                nc.scalar.copy(hatT[:, 0 : pw * 128], tr_pair[:, 0 : pw * 128])
                ps2_t = ps512p.tile([128, 512], dt.float32, tag="w512")
                ps2 = ps2_t[:, 0 : pw * 128]
                nc.tensor.matmul(ps2, lhsT=w_so[:], rhs=xt[:, 0 : pw * 128],
                                 start=True, stop=False)
                nc.tensor.matmul(ps2, lhsT=w_vo[:], rhs=hatT[:, 0 : pw * 128],
                                 start=False, stop=True)
                xb = winp.tile([128, 512], dt.float32, tag="xb")
                nc.vector.tensor_scalar(
                    out=xb[:, 0 : pw * 128], in0=xt[:, 0 : pw * 128],
                    scalar1=c_beff[:], scalar2=None, op0=mybir.AluOpType.add,
                )
                h3 = h3hold[:, 4 * k * 128 : (4 * k + pw) * 128]
                nc.vector.tensor_tensor(out=h3, in0=ps2, in1=xb[:, 0 : pw * 128],
                                        op=mybir.AluOpType.add)
                if k == npair_ - 1 and pad0:
                    nc.scalar.activation(
                        h3hold[:, (4 * k + pw - 1) * 128 + pad0 : (4 * k + pw) * 128],
                        xb[:, pad0:128],
                        mybir.ActivationFunctionType.Copy, scale=0.0,
                    )
                nc.vector.reduce_sum(sum1[:, k : k + 1], h3, axis=mybir.AxisListType.X)
                h3sq = winp.tile([128, 512], dt.float32, tag="h3sq")
                nc.scalar.activation(h3sq[:, 0 : pw * 128], h3,
                                     mybir.ActivationFunctionType.Square,
                                     accum_out=sq1[:, k : k + 1])

            # ---------------- AllReduce 1 ----------------
            st_sb = constp.tile([128, 2], dt.float32)
            nc.vector.reduce_sum(st_sb[:, 0:1], sum1[:], axis=mybir.AxisListType.X)
            nc.vector.reduce_sum(st_sb[:, 1:2], sq1[:], axis=mybir.AxisListType.X)
            nc.sync.dma_start(st1_in[:, :], st_sb[:])
            nc.gpsimd.collective_compute(
                "AllReduce", mybir.AluOpType.add, replica_groups=rg,
                ins=[st1_in[:, :].opt()], outs=[st1_out[:, :].opt()],
            )
            stg = constp.tile([128, 2], dt.float32)
            nc.sync.dma_start(stg[:], st1_out[:, :])
            s1c = constp.tile([128, 1], dt.float32)
            t1c = constp.tile([128, 1], dt.float32)
            _bn_coeffs(nc, mybir, smallp, stg, bc[:, 3:4], bc[:, 4:5], inv_n, s1c, t1c)

            # ------------- phase 2b: BN1 -> FFN -> h5 (node-tile pairs) ------
            starts = list(range(0, nt - 1, 2)) + [nt - 1]
            widths = [256] * ((nt - 1) // 2) + [128]
            for ip2, (t, wdt) in enumerate(zip(starts, widths)):
                cols = slice(t * 128, t * 128 + wdt)
                bnh = p2p.tile([128, 256], dt.bfloat16, tag="bnh")
                nc.vector.tensor_scalar(
                    out=bnh[:, 0:wdt], in0=h3hold[:, cols],
                    scalar1=s1c[:], scalar2=t1c[:],
                    op0=mybir.AluOpType.mult, op1=mybir.AluOpType.add,
                )
                if t == nt - 1 and pad0:
                    nc.scalar.activation(
                        bnh[:, pad0:128], bnh[:, pad0:128],
                        mybir.ActivationFunctionType.Copy, scale=0.0,
                    )
                f1a = ps512p.tile([128, 512], dt.float32, tag="w512")
                f1b = ps512p.tile([128, 512], dt.float32, tag="w512")
                nc.tensor.matmul(f1a[:, 0:wdt], lhsT=w_1b[:, 0:128], rhs=bnh[:, 0:wdt], start=True, stop=True)
                nc.tensor.matmul(f1b[:, 0:wdt], lhsT=w_1b[:, 128:256], rhs=bnh[:, 0:wdt], start=True, stop=True)
                ra = p2p.tile([128, 512], dt.bfloat16, tag="ra")
                nc.scalar.activation(
                    ra[:, 0:wdt], f1a[:, 0:wdt], mybir.ActivationFunctionType.Relu,
                    bias=bc[:, 0:1], scale=1.0,
                )
                nc.scalar.activation(
                    ra[:, 256 : 256 + wdt], f1b[:, 0:wdt], mybir.ActivationFunctionType.Relu,
                    bias=bc[:, 1:2], scale=1.0,
                )
                f2_t = ps512p.tile([128, 512], dt.float32, tag="w512")
                f2 = f2_t[:, 0:wdt]
                nc.tensor.matmul(f2, lhsT=w_2b[:, 0:128], rhs=ra[:, 0:wdt], start=True, stop=False)
                nc.tensor.matmul(f2, lhsT=w_2b[:, 128:256], rhs=ra[:, 256 : 256 + wdt], start=False, stop=True)
                f2b = p2p.tile([128, 256], dt.float32, tag="f2b")
                nc.vector.tensor_scalar(
                    out=f2b[:, 0:wdt], in0=f2,
                    scalar1=bc[:, 2:3], scalar2=None, op0=mybir.AluOpType.add,
                )
                h5 = h5hold[:, cols]
                nc.vector.tensor_tensor(out=h5, in0=f2b[:, 0:wdt], in1=bnh[:, 0:wdt], op=mybir.AluOpType.add)
                if t == nt - 1 and pad0:
                    nc.scalar.activation(
                        h5hold[:, t * 128 + pad0 : (t + 1) * 128],
                        f2b[:, pad0:128],
                        mybir.ActivationFunctionType.Copy, scale=0.0,
                    )
                nc.vector.reduce_sum(sum2[:, ip2 : ip2 + 1], h5, axis=mybir.AxisListType.X)
                h5sq = p2p.tile([128, 256], dt.float32, tag="h5sq")
                nc.scalar.activation(h5sq[:, 0:wdt], h5, mybir.ActivationFunctionType.Square,
                                     accum_out=sq2[:, ip2 : ip2 + 1])

            # ---------------- AllReduce 2 ----------------
            st_sb2 = constp.tile([128, 2], dt.float32)
            nc.vector.reduce_sum(st_sb2[:, 0:1], sum2[:], axis=mybir.AxisListType.X)
            nc.vector.reduce_sum(st_sb2[:, 1:2], sq2[:], axis=mybir.AxisListType.X)
            nc.sync.dma_start(st2_in[:, :], st_sb2[:])
            nc.gpsimd.collective_compute(
                "AllReduce", mybir.AluOpType.add, replica_groups=rg,
                ins=[st2_in[:, :].opt()], outs=[st2_out[:, :].opt()],
            )
            stg2 = constp.tile([128, 2], dt.float32)
            nc.sync.dma_start(stg2[:], st2_out[:, :])
            s2c = constp.tile([128, 1], dt.float32)
            t2c = constp.tile([128, 1], dt.float32)
            _bn_coeffs(nc, mybir, smallp, stg2, bc[:, 5:6], bc[:, 6:7], inv_n, s2c, t2c)

            # ---------------- phase 2c: y = BN2(h5) (pairs) ----------------
            for t, wdt in zip(starts, widths):
                cols = slice(t * 128, t * 128 + wdt)
                yt = p2p.tile([128, 256], dt.float32, tag="yt")
                nc.vector.tensor_scalar(
                    out=yt[:, 0:wdt], in0=h5hold[:, cols],
                    scalar1=s2c[:], scalar2=t2c[:],
                    op0=mybir.AluOpType.mult, op1=mybir.AluOpType.add,
                )
                nc.sync.dma_start(yT_out[:, cols], yt[:, 0:wdt])

    nc.finalize()
    return nc


def _bn_coeffs(nc, mybir, pool, stg, gcol, becol, inv_n, s_out, t_out):
    """From global (sum, sumsq) columns compute s = g*rstd, t = be - mu*s."""
    dt = mybir.dt
    mu = pool.tile([128, 1], dt.float32, tag="bn_mu")
    nc.scalar.activation(mu[:], stg[:, 0:1], mybir.ActivationFunctionType.Copy, scale=inv_n)
    e2 = pool.tile([128, 1], dt.float32, tag="bn_e2")
    nc.scalar.activation(e2[:], stg[:, 1:2], mybir.ActivationFunctionType.Copy, scale=inv_n)
    musq = pool.tile([128, 1], dt.float32, tag="bn_musq")
    nc.scalar.activation(musq[:], mu[:], mybir.ActivationFunctionType.Square)
    var = pool.tile([128, 1], dt.float32, tag="bn_var")
    nc.vector.tensor_tensor(out=var[:], in0=e2[:], in1=musq[:], op=mybir.AluOpType.subtract)
    varep = pool.tile([128, 1], dt.float32, tag="bn_varep")
    nc.vector.tensor_scalar_add(varep[:], var[:], EPS)
    sd = pool.tile([128, 1], dt.float32, tag="bn_sd")
    nc.scalar.activation(sd[:], varep[:], mybir.ActivationFunctionType.Sqrt)
    rstd = pool.tile([128, 1], dt.float32, tag="bn_rstd")
    nc.vector.reciprocal(rstd[:], sd[:])
    nc.vector.tensor_tensor(out=s_out[:], in0=gcol, in1=rstd[:], op=mybir.AluOpType.mult)
    mus = pool.tile([128, 1], dt.float32, tag="bn_mus")
    nc.vector.tensor_tensor(out=mus[:], in0=mu[:], in1=s_out[:], op=mybir.AluOpType.mult)
    nc.vector.tensor_tensor(out=t_out[:], in0=becol, in1=mus[:], op=mybir.AluOpType.subtract)


# ---------------------------------------------------------------------------
# Entry point
# ---------------------------------------------------------------------------

_CACHE = {}


def default_cfg():
    return {"n_nodes": N_NODES, "nc": NC, "nl": NL, "nlp": NLP}


def kernel(x, edge_index, Wq, bq, Wk, bk, Wv, bv, Ws, bs, WO, bO,
           W1, b1, W2, b2, g1, be1, g2, be2):
    from concourse.bass_utils import run_bass_kernel_spmd

    cfg = default_cfg()
    weights = {
        "Wq": Wq, "bq": bq, "Wk": Wk, "bk": bk, "Wv": Wv, "bv": bv,
        "Ws": Ws, "bs": bs, "WO": WO, "bO": bO, "W1": W1, "b1": b1,
        "W2": W2, "b2": b2, "g1": g1, "be1": be1, "g2": g2, "be2": be2,
    }
    in_maps, tev, tod = host_prep(np.asarray(x), np.asarray(edge_index), weights, cfg)

    key = (tev, tod)
    if key not in _CACHE:
        _CACHE[key] = build_kernel(cfg, tev, tod)
    nc = _CACHE[key]

    res = run_bass_kernel_spmd(nc, in_maps, core_ids=list(range(cfg["nc"])))
    outs = []
    for c in range(cfg["nc"]):
        yT = res.results[c]["yT_out"]
        outs.append(np.ascontiguousarray(yT.T[: cfg["nl"]]))
    return np.concatenate(outs, axis=0).astype(np.float32)
